# revision 26
# baseline (speedup 1.0000x reference)
"""CLUB loss kernel for Trainium2 (8 NeuronCores, SPMD row-sharded).

Math: the reference returns mean_i(pos_i - neg_i), a scalar.  Both the
pos and neg terms collapse into sums that never materialize the NxN
distance matrix:

  mean_pos = -0.5/N * (A - 2B + C)
      A = sum_{i,d} x[i,d]^2 * invv[i,d]
      B = sum_{i,d} x[i,d] * mu[i,d] * invv[i,d]
      C = sum_{i,d} mu[i,d]^2 * invv[i,d]
  mean_neg = -0.5 * (S_invv . S_x2 - 2 * S_muinvv . S_x + N*C) / N^2
      S_invv = sum_i invv[i,:]     S_muinvv = sum_i mu[i,:]*invv[i,:]
      S_x    = sum_j x[j,:]        S_x2     = sum_j x[j,:]^2
  loss = mean_pos - mean_neg

Each core handles 2048 rows (2 batches of x + matching mu/logvar rows)
and emits f32 partial sums; the host combines them in float64.

Layout: everything lives in the d-major layout (128, 1024): partition
q = (sub-slab b, dim d), free axis = row index within the sub-slab.
x arrives in this layout naturally (x[b] is (d, h*w) row-major); mu and
logvar are pre-transposed on the host as part of the shard layout.
With d on partitions every needed reduction is a free-axis row-sum, so
each quantity is one fused elementwise+accumulate instruction - no
on-chip transposes, no PSUM, no TensorEngine work at all (~20 compute
instructions per core).
"""

import sys

sys.path.insert(0, "/opt/trn_rl_repo")

import numpy as np
from contextlib import ExitStack

import concourse.bass as bass
import concourse.bacc as bacc
import concourse.tile as tile
from concourse import mybir
from concourse.bass_utils import run_bass_kernel_spmd

F32 = mybir.dt.float32
N_CORES = 8
B, D, H, W = 16, 64, 32, 32
HW = H * W                # 1024
N = B * HW                # 16384
NB = B // N_CORES         # 2 sub-slabs (batches) per core
ROWS = NB * HW            # 2048 rows per core
COLS = HW                 # free size of the (128, 1024) layout
# accum column map: quantity q, chunk c -> column q*NCH + c
QUANT = ["A", "B", "C", "Sx", "Sx2", "Sinvv", "Smuinvv"]
NCH = 4                   # accumulation chunks (bounds f32 chain length)
CW = COLS // NCH          # 256 columns per chunk


def build_nc() -> bass.Bass:
    nc = bacc.Bacc()
    xn = nc.dram_tensor("xn", [128, COLS], F32, kind="ExternalInput")
    mut = nc.dram_tensor("mut", [128, COLS], F32, kind="ExternalInput")
    lvt = nc.dram_tensor("lvt", [128, COLS], F32, kind="ExternalInput")
    accs = nc.dram_tensor("accs", [128, len(QUANT) * NCH], F32,
                          kind="ExternalOutput")

    with ExitStack() as ctx:
        tc = ctx.enter_context(tile.TileContext(nc))
        big = ctx.enter_context(tc.tile_pool(name="big", bufs=1))
        jp = ctx.enter_context(tc.tile_pool(name="jp", bufs=2))
        accp = ctx.enter_context(tc.tile_pool(name="accp", bufs=1))

        zerob = big.tile([128, 1], F32)
        nc.scalar.memzero(zerob[:])

        xb = big.tile([128, COLS], F32)
        mu = big.tile([128, COLS], F32)
        lv = big.tile([128, COLS], F32)
        # Split DMA issue across both HWDGE engines (SP + ACT), ordered by
        # when compute needs each chunk: lv heads the longest dependency
        # chain (exp -> muinvv -> B/C), mu is needed one stage later, x last.
        sls = [slice(h * CW, (h + 1) * CW) for h in range(NCH)]
        qs = [nc.sync, nc.scalar]
        for h in range(NCH):
            qs[h % 2].dma_start(out=lv[:, sls[h]], in_=lvt[:, sls[h]])
        for h in range(NCH):
            qs[h % 2].dma_start(out=mu[:, sls[h]], in_=mut[:, sls[h]])
        nc.sync.dma_start(out=xb[:, 0:COLS // 2], in_=xn[:, 0:COLS // 2])
        nc.scalar.dma_start(out=xb[:, COLS // 2:], in_=xn[:, COLS // 2:])

        invv = big.tile([128, COLS], F32)
        muinvv = big.tile([128, COLS], F32)
        x2 = big.tile([128, COLS], F32)
        acc = accp.tile([128, len(QUANT) * NCH], F32)

        def col(q, c):
            return acc[:, QUANT.index(q) * NCH + c:QUANT.index(q) * NCH + c + 1]

        M = mybir.AluOpType.mult

        def act(q, h, out, in_, func, scale=1.0):
            nc.scalar.activation(
                out=out, in_=in_, func=func, bias=zerob[:], scale=scale,
                accum_out=col(q, h),
            )

        def stt(q, h, in0, in1):
            jd = jp.tile([128, CW], F32, tag="jd", name=f"jd_{q}{h}")
            nc.vector.scalar_tensor_tensor(
                out=jd[:], in0=in0[:, sls[h]], scalar=1.0, in1=in1[:, sls[h]],
                op0=M, op1=M, accum_out=col(q, h),
            )

        def sx(h):
            jd = jp.tile([128, CW], F32, tag="jd", name=f"jd_sx{h}")
            nc.vector.tensor_scalar(
                out=jd[:], in0=xb[:, sls[h]], scalar1=1.0, scalar2=0.0,
                op0=M, op1=mybir.AluOpType.add, accum_out=col("Sx", h),
            )

        def smu(h):
            ja = jp.tile([128, CW], F32, tag="ja", name=f"ja_{h}")
            nc.scalar.activation(
                out=ja[:], in_=muinvv[:, sls[h]],
                func=mybir.ActivationFunctionType.Copy,
                bias=0.0, scale=1.0, accum_out=col("Smuinvv", h),
            )

        EXP = mybir.ActivationFunctionType.Exp
        SQ = mybir.ActivationFunctionType.Square

        # Emission order = engine program order.  Roll the
        # exp -> muinvv -> {B, C, Smuinvv} pipeline per quarter-chunk so
        # compute overlaps the remaining DMA transfers; x-gated work (x2,
        # Sx, A, B) naturally fills the tail as x arrives.
        for h in range(NCH):
            act("Sinvv", h, invv[:, sls[h]], lv[:, sls[h]], EXP, scale=-1.0)
            nc.gpsimd.tensor_mul(
                muinvv[:, sls[h]], mu[:, sls[h]], invv[:, sls[h]]
            )
            stt("C", h, mu, muinvv)
        for h in range(NCH):
            act("Sx2", h, x2[:, sls[h]], xb[:, sls[h]], SQ)
            sx(h)
            stt("B", h, xb, muinvv)
            stt("A", h, x2, invv)
            smu(h)

        nc.sync.dma_start(out=accs[:, :], in_=acc[:])
    return nc


_NC = None


def _get_nc():
    global _NC
    if _NC is None:
        _NC = build_nc()
        # bacc passes legalize multi-sync-wait instructions for TRN2 codegen
        _NC.compile()
    return _NC


def make_in_maps(x, mu, logvar):
    x = np.ascontiguousarray(np.asarray(x, dtype=np.float32))
    mu = np.asarray(mu, dtype=np.float32)
    lv = np.asarray(logvar, dtype=np.float32)
    in_maps = []
    for c in range(N_CORES):
        r0 = c * ROWS
        mu_t = np.concatenate(
            [mu[r0 + b * HW:r0 + (b + 1) * HW].T for b in range(NB)], axis=0
        )
        lv_t = np.concatenate(
            [lv[r0 + b * HW:r0 + (b + 1) * HW].T for b in range(NB)], axis=0
        )
        in_maps.append({
            "xn": x[c * NB:(c + 1) * NB].reshape(128, COLS),
            "mut": np.ascontiguousarray(mu_t),
            "lvt": np.ascontiguousarray(lv_t),
        })
    return in_maps


def combine(results) -> np.ndarray:
    nq = len(QUANT)
    tot = np.zeros((nq, 128), dtype=np.float64)
    for r in results:
        a = np.asarray(r["accs"], dtype=np.float64)  # (128, nq*NCH)
        for q in range(nq):
            tot[q] += a[:, q * NCH:(q + 1) * NCH].sum(axis=1)
    scal = {q: tot[i].sum() for i, q in enumerate(QUANT[:3])}
    vec = {q: tot[i].reshape(NB, D).sum(axis=0)
           for i, q in enumerate(QUANT) if i >= 3}
    A, Bs, C = scal["A"], scal["B"], scal["C"]
    mean_pos = -0.5 / N * (A - 2.0 * Bs + C)
    mean_D = (vec["Sinvv"] @ vec["Sx2"] - 2.0 * vec["Smuinvv"] @ vec["Sx"]
              + N * C) / float(N) ** 2
    loss = mean_pos + 0.5 * mean_D
    return np.array(loss, dtype=np.float32)


def kernel(x, mu, logvar, **_kwargs):
    nc = _get_nc()
    in_maps = make_in_maps(x, mu, logvar)
    res = run_bass_kernel_spmd(nc, in_maps, list(range(N_CORES)))
    return combine(res.results)


# revision 28
# speedup vs baseline: 1.0973x; 1.0973x over previous
"""CLUB loss kernel for Trainium2 (8 NeuronCores, SPMD row-sharded).

Math: the reference returns mean_i(pos_i - neg_i), a scalar.  Both the
pos and neg terms collapse into sums that never materialize the NxN
distance matrix:

  mean_pos = -0.5/N * (A - 2B + C)
      A = sum_{i,d} x[i,d]^2 * invv[i,d]
      B = sum_{i,d} x[i,d] * mu[i,d] * invv[i,d]
      C = sum_{i,d} mu[i,d]^2 * invv[i,d]
  mean_neg = -0.5 * (S_invv . S_x2 - 2 * S_muinvv . S_x + N*C) / N^2
      S_invv = sum_i invv[i,:]     S_muinvv = sum_i mu[i,:]*invv[i,:]
      S_x    = sum_j x[j,:]        S_x2     = sum_j x[j,:]^2
  loss = mean_pos - mean_neg

Each core handles 2048 rows (2 batches of x + matching mu/logvar rows)
and emits f32 partial sums; the host combines them in float64.

Layout: everything lives in the d-major layout (128, 1024): partition
q = (sub-slab b, dim d), free axis = row index within the sub-slab.
x arrives in this layout naturally (x[b] is (d, h*w) row-major); mu and
logvar are pre-transposed on the host as part of the shard layout.
With d on partitions every needed reduction is a free-axis row-sum, so
each quantity is one fused elementwise+accumulate instruction - no
on-chip transposes, no PSUM, no TensorEngine work at all (~20 compute
instructions per core).
"""

import sys

sys.path.insert(0, "/opt/trn_rl_repo")

import numpy as np
from contextlib import ExitStack

import concourse.bass as bass
import concourse.bacc as bacc
import concourse.tile as tile
from concourse import mybir
from concourse.bass_utils import run_bass_kernel_spmd

F32 = mybir.dt.float32
N_CORES = 8
B, D, H, W = 16, 64, 32, 32
HW = H * W                # 1024
N = B * HW                # 16384
NB = B // N_CORES         # 2 sub-slabs (batches) per core
ROWS = NB * HW            # 2048 rows per core
COLS = HW                 # free size of the (128, 1024) layout
# accum column map: quantity q, chunk c -> column q*NCH + c
QUANT = ["A", "B", "C", "Sx", "Sx2", "Sinvv", "Smuinvv"]
NCH = 2                   # accumulation chunks (bounds f32 chain length)
CW = COLS // NCH          # 512 columns per chunk


def build_nc() -> bass.Bass:
    nc = bacc.Bacc()
    xn = nc.dram_tensor("xn", [128, COLS], F32, kind="ExternalInput")
    mut = nc.dram_tensor("mut", [128, COLS], F32, kind="ExternalInput")
    lvt = nc.dram_tensor("lvt", [128, COLS], F32, kind="ExternalInput")
    accs = nc.dram_tensor("accs", [128, len(QUANT) * NCH], F32,
                          kind="ExternalOutput")

    with ExitStack() as ctx:
        tc = ctx.enter_context(tile.TileContext(nc))
        big = ctx.enter_context(tc.tile_pool(name="big", bufs=1))
        jp = ctx.enter_context(tc.tile_pool(name="jp", bufs=2))
        accp = ctx.enter_context(tc.tile_pool(name="accp", bufs=1))

        zerob = big.tile([128, 1], F32)
        nc.scalar.memzero(zerob[:])

        xb = big.tile([128, COLS], F32)
        mu = big.tile([128, COLS], F32)
        lv = big.tile([128, COLS], F32)
        # Split DMA issue across both HWDGE engines (SP + ACT), ordered by
        # when compute needs each chunk: lv heads the longest dependency
        # chain (exp -> muinvv -> B/C), mu is needed one stage later, x last.
        sls = [slice(h * CW, (h + 1) * CW) for h in range(NCH)]
        qs = [nc.sync, nc.scalar]
        for h in range(NCH):
            qs[h % 2].dma_start(out=lv[:, sls[h]], in_=lvt[:, sls[h]])
        for h in range(NCH):
            qs[h % 2].dma_start(out=mu[:, sls[h]], in_=mut[:, sls[h]])
        nc.sync.dma_start(out=xb[:, sls[0]], in_=xn[:, sls[0]])
        nc.scalar.dma_start(out=xb[:, sls[1]], in_=xn[:, sls[1]])

        invv = big.tile([128, COLS], F32)
        muinvv = big.tile([128, COLS], F32)
        x2 = big.tile([128, COLS], F32)
        acc = accp.tile([128, len(QUANT) * NCH], F32)

        def col(q, c):
            return acc[:, QUANT.index(q) * NCH + c:QUANT.index(q) * NCH + c + 1]

        M = mybir.AluOpType.mult

        def act(q, h, out, in_, func, scale=1.0):
            nc.scalar.activation(
                out=out, in_=in_, func=func, bias=zerob[:], scale=scale,
                accum_out=col(q, h),
            )

        def stt(q, h, in0, in1):
            jd = jp.tile([128, CW], F32, tag="jd", name=f"jd_{q}{h}")
            nc.vector.scalar_tensor_tensor(
                out=jd[:], in0=in0[:, sls[h]], scalar=1.0, in1=in1[:, sls[h]],
                op0=M, op1=M, accum_out=col(q, h),
            )

        def sx(h):
            jd = jp.tile([128, CW], F32, tag="jd", name=f"jd_sx{h}")
            nc.vector.tensor_scalar(
                out=jd[:], in0=xb[:, sls[h]], scalar1=1.0, scalar2=0.0,
                op0=M, op1=mybir.AluOpType.add, accum_out=col("Sx", h),
            )

        def smu(h):
            ja = jp.tile([128, CW], F32, tag="ja", name=f"ja_{h}")
            nc.scalar.activation(
                out=ja[:], in_=muinvv[:, sls[h]],
                func=mybir.ActivationFunctionType.Copy,
                bias=0.0, scale=1.0, accum_out=col("Smuinvv", h),
            )

        EXP = mybir.ActivationFunctionType.Exp
        SQ = mybir.ActivationFunctionType.Square

        # Emission order = engine program order.  Roll the
        # exp -> muinvv -> {B, C, Smuinvv} pipeline per quarter-chunk so
        # compute overlaps the remaining DMA transfers; x-gated work (x2,
        # Sx, A, B) naturally fills the tail as x arrives.
        for h in range(NCH):
            act("Sinvv", h, invv[:, sls[h]], lv[:, sls[h]], EXP, scale=-1.0)
            nc.gpsimd.tensor_mul(
                muinvv[:, sls[h]], mu[:, sls[h]], invv[:, sls[h]]
            )
            stt("C", h, mu, muinvv)
        for h in range(NCH):
            act("Sx2", h, x2[:, sls[h]], xb[:, sls[h]], SQ)
            sx(h)
            stt("B", h, xb, muinvv)
            stt("A", h, x2, invv)
            smu(h)

        nc.sync.dma_start(out=accs[:, :], in_=acc[:])
    return nc


_NC = None


def _get_nc():
    global _NC
    if _NC is None:
        _NC = build_nc()
        # bacc passes legalize multi-sync-wait instructions for TRN2 codegen
        _NC.compile()
    return _NC


def make_in_maps(x, mu, logvar):
    x = np.ascontiguousarray(np.asarray(x, dtype=np.float32))
    mu = np.asarray(mu, dtype=np.float32)
    lv = np.asarray(logvar, dtype=np.float32)
    in_maps = []
    for c in range(N_CORES):
        r0 = c * ROWS
        mu_t = np.concatenate(
            [mu[r0 + b * HW:r0 + (b + 1) * HW].T for b in range(NB)], axis=0
        )
        lv_t = np.concatenate(
            [lv[r0 + b * HW:r0 + (b + 1) * HW].T for b in range(NB)], axis=0
        )
        in_maps.append({
            "xn": x[c * NB:(c + 1) * NB].reshape(128, COLS),
            "mut": np.ascontiguousarray(mu_t),
            "lvt": np.ascontiguousarray(lv_t),
        })
    return in_maps


def combine(results) -> np.ndarray:
    nq = len(QUANT)
    tot = np.zeros((nq, 128), dtype=np.float64)
    for r in results:
        a = np.asarray(r["accs"], dtype=np.float64)  # (128, nq*NCH)
        for q in range(nq):
            tot[q] += a[:, q * NCH:(q + 1) * NCH].sum(axis=1)
    scal = {q: tot[i].sum() for i, q in enumerate(QUANT[:3])}
    vec = {q: tot[i].reshape(NB, D).sum(axis=0)
           for i, q in enumerate(QUANT) if i >= 3}
    A, Bs, C = scal["A"], scal["B"], scal["C"]
    mean_pos = -0.5 / N * (A - 2.0 * Bs + C)
    mean_D = (vec["Sinvv"] @ vec["Sx2"] - 2.0 * vec["Smuinvv"] @ vec["Sx"]
              + N * C) / float(N) ** 2
    loss = mean_pos + 0.5 * mean_D
    return np.array(loss, dtype=np.float32)


def kernel(x, mu, logvar, **_kwargs):
    nc = _get_nc()
    in_maps = make_in_maps(x, mu, logvar)
    res = run_bass_kernel_spmd(nc, in_maps, list(range(N_CORES)))
    return combine(res.results)
